# revision 17
# baseline (speedup 1.0000x reference)
"""PhotonicNeuralNetwork TRN2 kernel — 8-core data-parallel over batch, v2.

Architecture (all feature-major / transposed space, no device transposes):
  h.T = W @ x.T per layer; batch sharded 1024 rows/core across 8 cores.
  All matmul operands are bf16, host-precast (weights, tanh(x), noise, cm)
  — halves DMA vs fp32 and removes all on-device weight casts.

  Thermal path needs only ONE collective round: the layer-2 bias
  w2tn = W2 @ tn1 = (W2 @ Ks) @ t1 = M2 @ t1 must be applied inside
  tanh(z2 + b2 + w2tn) on device.  Each core: AllReduce t1 (8KB) ->
  computes a 256-row slice of M2 @ t1 with a fused DVE multiply-reduce ->
  AllGather slices (1KB/core).  Everything else thermal-related is
  batch-constant and applied on HOST: obias = Wout @ tn2 + bout with
  tn2 = 0.7*Ks@t1 + Ks@t2, where t1/t2 are host-summed per-core partial
  abs-sums (plain DMA exports, no second collective round).
  Dropped terms (validated 8.9e-5 rel in f64 sim): W2@(tn1*cm1),
  Wout@(tn2*cm2).

  Engine split: PE matmuls; Act tanh + PSUM evictions; Pool noise adds;
  DVE abs-reduces + fused (1+cm)*h ops + the thermal matvec.  L2 tail is
  emitted in m-halves interleaved with the L2 matmul stream so the
  tanh/noise/reduce/x3/y-matmul pipeline hides under PE compute.
"""
import os
import sys
import subprocess
import tempfile

import numpy as np

for _p in ("/opt/trn_rl_repo", "/root/.axon_site/_ro/trn_rl_repo"):
    if _p not in sys.path and os.path.isdir(_p):
        sys.path.append(_p)

import concourse.bass as bass  # noqa: E402
import concourse.mybir as mybir  # noqa: E402
import concourse.tile as tile  # noqa: E402
from concourse import bass_utils, bacc  # noqa: E402

# Problem shapes (hardcoded per contract)
B, D, H, DOUT = 8192, 1024, 2048, 2
N_CORES = 8
BC = B // N_CORES          # 1024 batch columns per core
SL = H // N_CORES          # 256 features per core for the matvec slice
KT1 = D // 128             # 8
MT = H // 128              # 16
KT2 = H // 128             # 16
NCH = BC // 512            # 2 psum chunks of 512
TN_SCALE = 0.05 * 0.3 * 0.05   # 7.5e-4, folded into Ks

SCHEME = os.environ.get("PNN_SCHEME", "fp8")
# bisection toggles (default = full-featured)
POOL_ADD = os.environ.get("PNN_POOL_ADD", "0") == "1"   # noise adds on Pool
USE_TTR = os.environ.get("PNN_TTR", "0") == "1"         # fused mult+reduce
WQ_SCALAR = os.environ.get("PNN_WQ", "scalar") == "scalar"  # w2 DMA queue

_CONSTS = {}
_NC_CACHE = {}


def _gen_constants():
    """Noise/coherence constants + crosstalk kernel, bit-exact with the
    reference's jax-on-CPU PRNG. Runs in a subprocess pinned to the CPU
    backend so the parent's jax platform config doesn't matter."""
    if _CONSTS:
        return _CONSTS
    script = r"""
import sys
import jax
jax.config.update("jax_platforms", "cpu")
import numpy as np
import jax.numpy as jnp
outdir = sys.argv[1]
B, H = 8192, 2048
nkey = jax.random.key(42)
for li in range(2):
    k_noise = jax.random.fold_in(nkey, 2 * li)
    k_phase = jax.random.fold_in(nkey, 2 * li + 1)
    n = jax.random.normal(k_noise, (B, H), jnp.float32) * np.float32(0.02)
    ph = jax.random.normal(k_phase, (B, H), jnp.float32) * np.float32(0.03)
    cm = (jnp.cos(ph) - np.float32(1.0)) * np.float32(0.03)
    np.save(f"{outdir}/n{li}.npy", np.asarray(n).T.copy())
    np.save(f"{outdir}/cm{li}.npy", np.asarray(cm).T.copy())
idx = jnp.arange(H, dtype=jnp.float32)
dist = jnp.abs(idx[:, None] - idx[None, :])
K = jnp.where(dist > 0, 1.0 / (dist * dist), 0.0)
np.save(f"{outdir}/K.npy", np.asarray(K))
"""
    with tempfile.TemporaryDirectory() as td:
        env = dict(os.environ)
        env["JAX_PLATFORMS"] = "cpu"
        env.pop("JAX_PLATFORM_NAME", None)
        subprocess.run([sys.executable, "-c", script, td], env=env, check=True,
                       capture_output=True)
        for li in range(2):
            _CONSTS[f"noise{li}T"] = np.load(f"{td}/n{li}.npy")   # [H, B] fp32
            _CONSTS[f"cmm{li}T"] = np.load(f"{td}/cm{li}.npy")    # [H, B] fp32
        K = np.load(f"{td}/K.npy")                                # [H, H] fp32
    _CONSTS["Ks"] = (K.astype(np.float64) * TN_SCALE).astype(np.float32)
    return _CONSTS


def _build_nc(scheme):
    key = (scheme, POOL_ADD, USE_TTR, WQ_SCALAR)
    if key in _NC_CACHE:
        return _NC_CACHE[key]
    bf16 = mybir.dt.bfloat16
    fp8 = mybir.dt.float8e4
    f32 = mybir.dt.float32
    ACT = mybir.ActivationFunctionType
    ALU = mybir.AluOpType
    FP8 = scheme == "fp8"
    MMDT = fp8 if FP8 else bf16         # matmul operand dtype (hidden layers)
    DR = mybir.MatmulPerfMode.DoubleRow if FP8 else None
    KP1 = KT1 // 2 if FP8 else KT1      # lhs k-tile count layer 1
    KP2 = KT2 // 2 if FP8 else KT2      # lhs k-tile count layer 2
    WSC = 0.125 if FP8 else 1.0         # un-scale for the x8 fp8 weights
    WSC2 = 0.125 / 64.0 if FP8 else 1.0  # also undo the x64 cm/x2 scale
    KW = 2 * H if FP8 else H            # weight tile free size
    KX = 2 * BC if FP8 else BC          # x tile free size

    nc = bacc.Bacc(trn_type="TRN2", target_bir_lowering=False, debug=False,
                   num_devices=N_CORES)

    x0T_d = nc.dram_tensor("x0T", [KP1 * 128, KX], MMDT, kind="ExternalInput")
    w1T_d = nc.dram_tensor("w1T", [KP1 * 128, KW], MMDT, kind="ExternalInput")
    w2T_d = nc.dram_tensor("w2T", [KP2 * 128, KW], MMDT, kind="ExternalInput")
    woutT_d = nc.dram_tensor("woutT", [H, DOUT], bf16, kind="ExternalInput")
    m2T_d = nc.dram_tensor("m2T", [H, H], fp8, kind="ExternalInput")
    NDT = fp8 if FP8 else bf16          # noise/cm storage dtype
    NSC = 64.0 if FP8 else 1.0          # noise/cm host-side scale
    n1T_d = nc.dram_tensor("n1T", [H, BC], NDT, kind="ExternalInput")
    cm1T_d = nc.dram_tensor("cm1T", [H, BC], NDT, kind="ExternalInput")
    n2T_d = nc.dram_tensor("n2T", [H, BC], NDT, kind="ExternalInput")
    cm2T_d = nc.dram_tensor("cm2T", [H, BC], NDT, kind="ExternalInput")
    b1_d = nc.dram_tensor("b1s", [128, MT], f32, kind="ExternalInput")
    b2_d = nc.dram_tensor("b2s", [128, MT], f32, kind="ExternalInput")
    y_d = nc.dram_tensor("y", [DOUT, BC], f32, kind="ExternalOutput")
    t1p_d = nc.dram_tensor("t1p", [H], f32, kind="ExternalOutput")
    t2p_d = nc.dram_tensor("t2p", [H], f32, kind="ExternalOutput")
    DEBUG = os.environ.get("PNN_DEBUG", "0") == "1"
    if DEBUG:
        dbg_d = {n: nc.dram_tensor(n, [128, MT], f32, kind="ExternalOutput")
                 for n in ["d_t1", "d_w2tn", "d_bias2"]}

    def mm_ap(t, sl):
        if FP8:
            return t[:].rearrange("p (i f) -> p i f", i=2)[:, :, sl]
        return t[:, sl]

    MMKW = {"perf_mode": DR} if FP8 else {}

    RG = [list(range(N_CORES))]
    with tile.TileContext(nc) as tc:
        with tc.tile_pool(name="dram", bufs=1, space="DRAM") as dram, \
             tc.tile_pool(name="smalls", bufs=1) as smalls, \
             tc.tile_pool(name="psum_mm", bufs=4, space="PSUM") as psum_mm, \
             tc.tile_pool(name="psum_out", bufs=2, space="PSUM") as psum_out, \
             tc.tile_pool(name="xx", bufs=1) as xx_pool, \
             tc.tile_pool(name="w2h", bufs=1) as w2h_pool, \
             tc.tile_pool(name="w2h2", bufs=1) as _unused_pool:

            # --- small persistent tiles ---
            b1_sb = smalls.tile([128, MT], f32)
            b2_sb = smalls.tile([128, MT], f32)
            t1_sb = smalls.tile([128, MT], f32)
            t2_sb = smalls.tile([128, MT], f32)
            w2tn_sb = smalls.tile([128, MT], f32)
            bias2_sb = smalls.tile([128, MT], f32)
            t1b_sb = smalls.tile([128, MT], fp8)
            w2tnp_sb = smalls.tile([1, H], f32)
            woutm_sb = smalls.tile([128, KT2 * DOUT], bf16)

            nc.sync.dma_start(b1_sb[:], b1_d.ap()[:])
            nc.sync.dma_start(b2_sb[:], b2_d.ap()[:])
            wout_r = woutT_d.ap().rearrange("(k p) o -> p k o", p=128)
            nc.sync.dma_start(
                woutm_sb[:].rearrange("p (k o) -> p k o", k=KT2), wout_r)

            # --- DRAM bounce buffers: single AllReduce of the per-core
            #     w2tn partials (each core PE-matvecs M2T @ its local t1) ---
            wb = dram.tile([H], f32)
            wr = dram.tile([H], f32)

            x2 = []
            with tc.tile_pool(name="x0", bufs=1) as x0_pool, \
                 tc.tile_pool(name="w1", bufs=1) as w1_pool, \
                 tc.tile_pool(name="h1c", bufs=6 if FP8 else 3) as h1_pool, \
                 tc.tile_pool(name="nz1", bufs=3 if FP8 else 2) as nz1_pool, \
                 tc.tile_pool(name="habs", bufs=2) as habs_pool, \
                 tc.tile_pool(name="m2tp", bufs=1) as m2t_pool, \
                 tc.tile_pool(name="cm1", bufs=3 if FP8 else 2) as cm1_pool:

                # ---- interleaved x0/w1 loads so the first psum chain can
                #      start after ~1MB of DMA ----
                x0, w1 = [], []
                for kt in range(KP1):
                    x0t = x0_pool.tile([128, KX], MMDT, name=f"x0_{kt}")
                    nc.sync.dma_start(x0t[:], x0T_d.ap()[bass.ts(kt, 128), :])
                    x0.append(x0t)
                    w1t = w1_pool.tile([128, KW], MMDT, name=f"w1_{kt}")
                    nc.sync.dma_start(w1t[:], w1T_d.ap()[bass.ts(kt, 128), :])
                    w1.append(w1t)

                # --- W2 + M2 slices emitted AFTER x0/w1 so the startup
                #     loads win the DMA-queue race ---
                w2 = []
                for kt in range(KP2):
                    w2t = w2h_pool.tile([128, KW], MMDT, name=f"w2_{kt}")
                    wdma = (nc.scalar.dma_start if WQ_SCALAR
                            else nc.sync.dma_start)
                    wdma(w2t[:], w2T_d.ap()[bass.ts(kt, 128), :])
                    w2.append(w2t)
                m2t = []
                for kt in range(MT):
                    m2tt = m2t_pool.tile([128, H], fp8, name=f"m2t_{kt}")
                    wdma = (nc.scalar.dma_start if WQ_SCALAR
                            else nc.sync.dma_start)
                    wdma(m2tt[:], m2T_d.ap()[bass.ts(kt, 128), :])
                    m2t.append(m2tt)

                # ---- L1: matmuls, tanh+bias, +noise (Pool), abs-reduce
                #      (DVE), x2 = (1+cm1)*h1 fused on DVE ----
                for mt in range(MT):
                    h1t = h1_pool.tile([128, BC], bf16, name="h1c")
                    pss = [psum_mm.tile([128, 512], f32, name="psmm")
                           for _ in range(NCH)]
                    for kt in range(KP1):
                        for nch in range(NCH):
                            nc.tensor.matmul(
                                pss[nch][:], mm_ap(w1[kt], bass.ts(mt, 128)),
                                mm_ap(x0[kt], bass.ts(nch, 512)),
                                start=(kt == 0), stop=(kt == KP1 - 1), **MMKW)
                    for nch in range(NCH):
                        nc.scalar.activation(h1t[:, bass.ts(nch, 512)],
                                             pss[nch][:], ACT.Tanh,
                                             bias=b1_sb[:, mt:mt + 1],
                                             scale=WSC)
                    nzt = nz1_pool.tile([128, BC], NDT, name="nz1")
                    nc.sync.dma_start(nzt[:], n1T_d.ap()[bass.ts(mt, 128), :])
                    eng_add = nc.gpsimd if POOL_ADD else nc.vector
                    if FP8:
                        eng_add.scalar_tensor_tensor(
                            out=h1t[:], in0=nzt[:], scalar=1.0 / NSC,
                            in1=h1t[:], op0=ALU.mult, op1=ALU.add)
                    else:
                        eng_add.tensor_tensor(out=h1t[:], in0=h1t[:],
                                              in1=nzt[:], op=ALU.add)
                    habs = habs_pool.tile([128, BC], bf16, name="habs")
                    nc.scalar.activation(habs[:], h1t[:], ACT.Abs,
                                         accum_out=t1_sb[:, mt:mt + 1])
                    cmt = cm1_pool.tile([128, BC], NDT, name="cm1")
                    nc.sync.dma_start(cmt[:], cm1T_d.ap()[bass.ts(mt, 128), :])
                    if FP8:
                        if mt % 2 == 0:
                            x2.append(xx_pool.tile([128, KX], fp8,
                                                   name=f"x2_{mt // 2}"))
                        x2dst = x2[mt // 2][:, bass.ts(mt % 2, BC)]
                    else:
                        x2.append(xx_pool.tile([128, BC], bf16,
                                               name=f"x2_{mt}"))
                        x2dst = x2[mt][:]
                    nc.vector.scalar_tensor_tensor(
                        out=x2dst, in0=cmt[:], scalar=NSC, in1=h1t[:],
                        op0=ALU.add, op1=ALU.mult)

                # ---- thermal round: export t1 partial, AllReduce t1,
                #      fused DVE matvec slice of M2@t1, AllGather ----
                t1p_r = t1p_d.ap().rearrange("(m p) -> p m", p=128)
                nc.sync.dma_start(t1p_r, t1_sb[:])
                # local w2tn partial = M2T.T @ t1_local on the PE, then ONE
                # 8KB AllReduce sums the partials across cores
                nc.vector.tensor_scalar_mul(t1b_sb[:], t1_sb[:], 0.5)
                for ch in range(4):
                    psmv = psum_out.tile([1, 512], f32, name="psmv")
                    for kt in range(MT):
                        nc.tensor.matmul(
                            psmv[:], t1b_sb[:, kt:kt + 1],
                            m2t[kt][:, bass.ts(ch, 512)],
                            start=(kt == 0), stop=(kt == MT - 1))
                    nc.scalar.mul(w2tnp_sb[:, bass.ts(ch, 512)], psmv[:],
                                  1.0 / 16384.0)
                wb_r = wb.rearrange("(a m) -> a m", a=1)
                nc.sync.dma_start(wb_r, w2tnp_sb[:])
                nc.gpsimd.collective_compute(
                    "AllReduce", ALU.add, replica_groups=RG,
                    ins=[wb.opt()], outs=[wr.opt()])
                wr_r = wr.rearrange("(m p) -> p m", p=128)
                nc.sync.dma_start(w2tn_sb[:], wr_r)
                nc.vector.tensor_tensor(out=bias2_sb[:], in0=b2_sb[:],
                                        in1=w2tn_sb[:], op=ALU.add)

            # ---- L2: full-K psum accumulation, Act-copy eviction to bf16
            #      z2; tail (tanh+bias2, +noise, reduce, x3, y-matmul)
            #      emitted per m-half so it pipelines under the matmuls ----
            with tc.tile_pool(name="z2p", bufs=1) as z2_pool, \
                 tc.tile_pool(name="x3c", bufs=1) as x3_pool, \
                 tc.tile_pool(name="nz2", bufs=3) as nz2_pool, \
                 tc.tile_pool(name="cm2", bufs=3) as cm2_pool, \
                 tc.tile_pool(name="ysb", bufs=1) as y_pool:

                z2 = [None] * MT
                x3 = [None] * MT
                pso = [psum_out.tile([DOUT, 512], f32, name="pso")
                       for _ in range(NCH)]

                def emit_mains(mts):
                    for mt in mts:
                        z2t = z2_pool.tile([128, BC], bf16, name=f"z2_{mt}")
                        z2[mt] = z2t
                        for nch in range(NCH):
                            ps = psum_mm.tile([128, 512], f32, name="psmm")
                            for kt in range(KP2):
                                nc.tensor.matmul(
                                    ps[:], mm_ap(w2[kt], bass.ts(mt, 128)),
                                    mm_ap(x2[kt], bass.ts(nch, 512)),
                                    start=(kt == 0), stop=(kt == KP2 - 1),
                                    **MMKW)
                            nc.scalar.mul(z2t[:, bass.ts(nch, 512)], ps[:],
                                          WSC2)

                def emit_tail(mts):
                    for mt in mts:
                        z2t = z2[mt]
                        for nch in range(NCH):
                            dst = z2t[:, bass.ts(nch, 512)]
                            nc.scalar.activation(dst, dst, ACT.Tanh,
                                                 bias=bias2_sb[:, mt:mt + 1])
                        nzt = nz2_pool.tile([128, BC], NDT, name="nz2")
                        nc.sync.dma_start(nzt[:],
                                          n2T_d.ap()[bass.ts(mt, 128), :])
                        eng_add = nc.gpsimd if POOL_ADD else nc.vector
                        if FP8:
                            eng_add.scalar_tensor_tensor(
                                out=z2t[:], in0=nzt[:], scalar=1.0 / NSC,
                                in1=z2t[:], op0=ALU.mult, op1=ALU.add)
                        else:
                            eng_add.tensor_tensor(out=z2t[:], in0=z2t[:],
                                                  in1=nzt[:], op=ALU.add)
                        nc.vector.tensor_reduce(
                            out=t2_sb[:, mt:mt + 1], in_=z2t[:],
                            axis=mybir.AxisListType.X, op=ALU.add,
                            apply_absolute_value=True)
                        cmt = cm2_pool.tile([128, BC], NDT, name="cm2")
                        nc.sync.dma_start(cmt[:],
                                          cm2T_d.ap()[bass.ts(mt, 128), :])
                        x3t = x3_pool.tile([128, BC], bf16, name=f"x3_{mt}")
                        nc.vector.scalar_tensor_tensor(
                            out=x3t[:], in0=cmt[:], scalar=NSC, in1=z2t[:],
                            op0=ALU.add, op1=ALU.mult)
                        x3[mt] = x3t

                def emit_ymm(kts):
                    for kt in kts:
                        for nch in range(NCH):
                            nc.tensor.matmul(
                                pso[nch][:], woutm_sb[:, bass.ts(kt, DOUT)],
                                x3[kt][:, bass.ts(nch, 512)],
                                start=(kt == 0), stop=(kt == KT2 - 1))

                HALF = MT // 2
                emit_mains(range(0, HALF))
                emit_tail(range(0, HALF))          # gated on bias2; Act does
                emit_mains(range(HALF, MT))        # these tanhs between the
                emit_ymm(range(0, HALF))           # second-half evictions
                emit_tail(range(HALF, MT))
                emit_ymm(range(HALF, MT))

                # ---- t2 partial export + output eviction ----
                t2p_r = t2p_d.ap().rearrange("(m p) -> p m", p=128)
                nc.sync.dma_start(t2p_r, t2_sb[:])
                y_sb = y_pool.tile([DOUT, BC], f32, name="ysb")
                for nch in range(NCH):
                    nc.scalar.copy(y_sb[:, bass.ts(nch, 512)], pso[nch][:])
                nc.sync.dma_start(y_d.ap()[:], y_sb[:])
                if DEBUG:
                    for nm, t in [("d_t1", t1_sb), ("d_w2tn", w2tn_sb),
                                  ("d_bias2", bias2_sb)]:
                        nc.sync.dma_start(dbg_d[nm].ap()[:], t[:])

    nc.finalize()
    _NC_CACHE[(scheme, POOL_ADD, USE_TTR, WQ_SCALAR)] = nc
    return nc


def _prep_inputs(x, W1, b1, W2, b2, Wout, bout, scheme):
    consts = _gen_constants()
    import ml_dtypes
    f32 = np.float32
    bf = np.dtype(ml_dtypes.bfloat16)

    FP8 = scheme == "fp8"
    f8 = np.dtype(ml_dtypes.float8_e4m3)

    def pair_rows(A):
        """[R, C] -> [R//2, 2C]: row (ktp*128+p) = [A[ktp*256+p], A[ktp*256+128+p]]
        — the DoubleRow paired-k layout."""
        R, C = A.shape
        return np.ascontiguousarray(
            A.reshape(R // 256, 2, 128, C).transpose(0, 2, 1, 3)
            .reshape(R // 2, 2 * C))

    x0Tf = np.tanh(np.asarray(x, f32)).T                       # [D, B] f32
    if FP8:
        W1T = pair_rows(np.asarray(W1, f32).T * 8).astype(f8)  # [D/2, 2H]
        W2T = pair_rows(np.asarray(W2, f32).T * 8).astype(f8)  # [H/2, 2H]
    else:
        W1T = np.asarray(W1, f32).T.astype(bf)                 # [D, H]
        W2T = np.asarray(W2, f32).T.astype(bf)                 # [H, H]
    WoutT = (np.asarray(Wout, f32).T
             * (1.0 / 64.0 if FP8 else 1.0)).astype(bf)        # [H, 2]
    Ks64 = consts["Ks"].astype(np.float64)
    M2T = np.ascontiguousarray(
        (np.asarray(W2, np.float64) @ Ks64).T * 32768.0).astype(f8)  # [H, H]
    b1s = np.ascontiguousarray(np.asarray(b1, f32).reshape(MT, 128).T)
    b2s = np.ascontiguousarray(np.asarray(b2, f32).reshape(MT, 128).T)
    ndt = f8 if FP8 else bf
    nsc = np.float32(64.0 if FP8 else 1.0)
    n1T = (consts["noise0T"] * nsc).astype(ndt)
    cm1T = (consts["cmm0T"] * nsc).astype(ndt)
    n2T = (consts["noise1T"] * nsc).astype(ndt)
    cm2T = (consts["cmm1T"] * nsc).astype(ndt)

    in_maps = []
    for c in range(N_CORES):
        bs = slice(c * BC, (c + 1) * BC)
        fs = slice(c * SL, (c + 1) * SL)
        if FP8:
            x0c = pair_rows(np.ascontiguousarray(x0Tf[:, bs])).astype(f8)
        else:
            x0c = np.ascontiguousarray(x0Tf[:, bs]).astype(bf)
        in_maps.append({
            "x0T": x0c,
            "w1T": W1T,
            "w2T": W2T,
            "woutT": WoutT,
            "m2T": M2T,
            "n1T": np.ascontiguousarray(n1T[:, bs]),
            "cm1T": np.ascontiguousarray(cm1T[:, bs]),
            "n2T": np.ascontiguousarray(n2T[:, bs]),
            "cm2T": np.ascontiguousarray(cm2T[:, bs]),
            "b1s": b1s,
            "b2s": b2s,
        })
    return in_maps


def kernel(x, W1, b1, W2, b2, Wout, bout, **kw):
    scheme = SCHEME
    nc = _build_nc(scheme)
    in_maps = _prep_inputs(x, W1, b1, W2, b2, Wout, bout, scheme)
    res = bass_utils.run_bass_kernel_spmd(nc, in_maps,
                                          core_ids=list(range(N_CORES)))
    # host adds the batch-constant output offset: Wout@tn2 + bout, with
    # tn2 = 0.7*Ks@t1 + Ks@t2 and t1/t2 summed from per-core partials
    consts = _gen_constants()
    MoutR = (np.asarray(Wout, np.float64)
             @ consts["Ks"].astype(np.float64))
    t1f = np.zeros(H, np.float64)
    t2f = np.zeros(H, np.float64)
    for c in range(N_CORES):
        t1f += res.results[c]["t1p"].astype(np.float64)
        t2f += res.results[c]["t2p"].astype(np.float64)
    obias = (0.7 * (MoutR @ t1f) + MoutR @ t2f
             + np.asarray(bout, np.float64)).astype(np.float32)
    out = np.empty((B, DOUT), np.float32)
    for c in range(N_CORES):
        out[c * BC:(c + 1) * BC, :] = res.results[c]["y"].T + obias[None, :]
    return out


# revision 18
# speedup vs baseline: 1.1718x; 1.1718x over previous
"""PhotonicNeuralNetwork TRN2 kernel — 8-core data-parallel over batch, v2.

Architecture (all feature-major / transposed space, no device transposes):
  h.T = W @ x.T per layer; batch sharded 1024 rows/core across 8 cores.
  All matmul operands are bf16, host-precast (weights, tanh(x), noise, cm)
  — halves DMA vs fp32 and removes all on-device weight casts.

  Thermal path needs only ONE collective round: the layer-2 bias
  w2tn = W2 @ tn1 = (W2 @ Ks) @ t1 = M2 @ t1 must be applied inside
  tanh(z2 + b2 + w2tn) on device.  Each core: AllReduce t1 (8KB) ->
  computes a 256-row slice of M2 @ t1 with a fused DVE multiply-reduce ->
  AllGather slices (1KB/core).  Everything else thermal-related is
  batch-constant and applied on HOST: obias = Wout @ tn2 + bout with
  tn2 = 0.7*Ks@t1 + Ks@t2, where t1/t2 are host-summed per-core partial
  abs-sums (plain DMA exports, no second collective round).
  Dropped terms (validated 8.9e-5 rel in f64 sim): W2@(tn1*cm1),
  Wout@(tn2*cm2).

  Engine split: PE matmuls; Act tanh + PSUM evictions; Pool noise adds;
  DVE abs-reduces + fused (1+cm)*h ops + the thermal matvec.  L2 tail is
  emitted in m-halves interleaved with the L2 matmul stream so the
  tanh/noise/reduce/x3/y-matmul pipeline hides under PE compute.
"""
import os
import sys
import subprocess
import tempfile

import numpy as np

for _p in ("/opt/trn_rl_repo", "/root/.axon_site/_ro/trn_rl_repo"):
    if _p not in sys.path and os.path.isdir(_p):
        sys.path.append(_p)

import concourse.bass as bass  # noqa: E402
import concourse.mybir as mybir  # noqa: E402
import concourse.tile as tile  # noqa: E402
from concourse import bass_utils, bacc  # noqa: E402

# Problem shapes (hardcoded per contract)
B, D, H, DOUT = 8192, 1024, 2048, 2
N_CORES = 8
BC = B // N_CORES          # 1024 batch columns per core
SL = H // N_CORES          # 256 features per core for the matvec slice
KT1 = D // 128             # 8
MT = H // 128              # 16
KT2 = H // 128             # 16
NCH = BC // 512            # 2 psum chunks of 512
TN_SCALE = 0.05 * 0.3 * 0.05   # 7.5e-4, folded into Ks

SCHEME = os.environ.get("PNN_SCHEME", "fp8")
# bisection toggles (default = full-featured)
POOL_ADD = os.environ.get("PNN_POOL_ADD", "0") == "1"   # noise adds on Pool
USE_TTR = os.environ.get("PNN_TTR", "0") == "1"         # fused mult+reduce
WQ_SCALAR = os.environ.get("PNN_WQ", "scalar") == "scalar"  # w2 DMA queue

_CONSTS = {}
_NC_CACHE = {}


def _gen_constants():
    """Noise/coherence constants + crosstalk kernel, bit-exact with the
    reference's jax-on-CPU PRNG. Runs in a subprocess pinned to the CPU
    backend so the parent's jax platform config doesn't matter."""
    if _CONSTS:
        return _CONSTS
    script = r"""
import sys
import jax
jax.config.update("jax_platforms", "cpu")
import numpy as np
import jax.numpy as jnp
outdir = sys.argv[1]
B, H = 8192, 2048
nkey = jax.random.key(42)
for li in range(2):
    k_noise = jax.random.fold_in(nkey, 2 * li)
    k_phase = jax.random.fold_in(nkey, 2 * li + 1)
    n = jax.random.normal(k_noise, (B, H), jnp.float32) * np.float32(0.02)
    ph = jax.random.normal(k_phase, (B, H), jnp.float32) * np.float32(0.03)
    cm = (jnp.cos(ph) - np.float32(1.0)) * np.float32(0.03)
    np.save(f"{outdir}/n{li}.npy", np.asarray(n).T.copy())
    np.save(f"{outdir}/cm{li}.npy", np.asarray(cm).T.copy())
idx = jnp.arange(H, dtype=jnp.float32)
dist = jnp.abs(idx[:, None] - idx[None, :])
K = jnp.where(dist > 0, 1.0 / (dist * dist), 0.0)
np.save(f"{outdir}/K.npy", np.asarray(K))
"""
    with tempfile.TemporaryDirectory() as td:
        env = dict(os.environ)
        env["JAX_PLATFORMS"] = "cpu"
        env.pop("JAX_PLATFORM_NAME", None)
        subprocess.run([sys.executable, "-c", script, td], env=env, check=True,
                       capture_output=True)
        for li in range(2):
            _CONSTS[f"noise{li}T"] = np.load(f"{td}/n{li}.npy")   # [H, B] fp32
            _CONSTS[f"cmm{li}T"] = np.load(f"{td}/cm{li}.npy")    # [H, B] fp32
        K = np.load(f"{td}/K.npy")                                # [H, H] fp32
    _CONSTS["Ks"] = (K.astype(np.float64) * TN_SCALE).astype(np.float32)
    return _CONSTS


def _build_nc(scheme):
    key = (scheme, POOL_ADD, USE_TTR, WQ_SCALAR)
    if key in _NC_CACHE:
        return _NC_CACHE[key]
    bf16 = mybir.dt.bfloat16
    fp8 = mybir.dt.float8e4
    f32 = mybir.dt.float32
    ACT = mybir.ActivationFunctionType
    ALU = mybir.AluOpType
    FP8 = scheme == "fp8"
    MMDT = fp8 if FP8 else bf16         # matmul operand dtype (hidden layers)
    DR = mybir.MatmulPerfMode.DoubleRow if FP8 else None
    KP1 = KT1 // 2 if FP8 else KT1      # lhs k-tile count layer 1
    KP2 = KT2 // 2 if FP8 else KT2      # lhs k-tile count layer 2
    WSC = 0.125 if FP8 else 1.0         # un-scale for the x8 fp8 weights
    WSC2 = 0.125 / 64.0 if FP8 else 1.0  # also undo the x64 cm/x2 scale
    KW = 2 * H if FP8 else H            # weight tile free size
    KX = 2 * BC if FP8 else BC          # x tile free size

    nc = bacc.Bacc(trn_type="TRN2", target_bir_lowering=False, debug=False,
                   num_devices=N_CORES)

    x0T_d = nc.dram_tensor("x0T", [KP1 * 128, KX], MMDT, kind="ExternalInput")
    w1T_d = nc.dram_tensor("w1T", [KP1 * 128, KW], MMDT, kind="ExternalInput")
    w2T_d = nc.dram_tensor("w2T", [KP2 * 128, KW], MMDT, kind="ExternalInput")
    woutT_d = nc.dram_tensor("woutT", [H, DOUT], bf16, kind="ExternalInput")
    m2T_d = nc.dram_tensor("m2T", [H, H], fp8, kind="ExternalInput")
    NDT = fp8 if FP8 else bf16          # noise/cm storage dtype
    NSC = 64.0 if FP8 else 1.0          # noise/cm host-side scale
    n1T_d = nc.dram_tensor("n1T", [H, BC], NDT, kind="ExternalInput")
    cm1T_d = nc.dram_tensor("cm1T", [H, BC], NDT, kind="ExternalInput")
    n2T_d = nc.dram_tensor("n2T", [H, BC], NDT, kind="ExternalInput")
    cm2T_d = nc.dram_tensor("cm2T", [H, BC], NDT, kind="ExternalInput")
    b1_d = nc.dram_tensor("b1s", [128, MT], f32, kind="ExternalInput")
    b2_d = nc.dram_tensor("b2s", [128, MT], f32, kind="ExternalInput")
    y_d = nc.dram_tensor("y", [DOUT, BC], f32, kind="ExternalOutput")
    t1p_d = nc.dram_tensor("t1p", [H], f32, kind="ExternalOutput")
    t2p_d = nc.dram_tensor("t2p", [H], f32, kind="ExternalOutput")
    DEBUG = os.environ.get("PNN_DEBUG", "0") == "1"
    if DEBUG:
        dbg_d = {n: nc.dram_tensor(n, [128, MT], f32, kind="ExternalOutput")
                 for n in ["d_t1", "d_w2tn", "d_bias2"]}

    def mm_ap(t, sl):
        if FP8:
            return t[:].rearrange("p (i f) -> p i f", i=2)[:, :, sl]
        return t[:, sl]

    MMKW = {"perf_mode": DR} if FP8 else {}

    RG = [list(range(N_CORES))]
    with tile.TileContext(nc) as tc:
        with tc.tile_pool(name="dram", bufs=1, space="DRAM") as dram, \
             tc.tile_pool(name="smalls", bufs=1) as smalls, \
             tc.tile_pool(name="psum_mm", bufs=6, space="PSUM") as psum_mm, \
             tc.tile_pool(name="psum_out", bufs=2, space="PSUM") as psum_out, \
             tc.tile_pool(name="xx", bufs=1) as xx_pool, \
             tc.tile_pool(name="w2h", bufs=1) as w2h_pool, \
             tc.tile_pool(name="w2h2", bufs=1) as _unused_pool:

            # --- small persistent tiles ---
            b1_sb = smalls.tile([128, MT], f32)
            b2_sb = smalls.tile([128, MT], f32)
            t1_sb = smalls.tile([128, MT], f32)
            t2_sb = smalls.tile([128, MT], f32)
            w2tn_sb = smalls.tile([128, MT], f32)
            bias2_sb = smalls.tile([128, MT], f32)
            t1b_sb = smalls.tile([128, MT], fp8)
            w2tnp_sb = smalls.tile([1, H], f32)
            woutm_sb = smalls.tile([128, KT2 * DOUT], bf16)

            nc.sync.dma_start(b1_sb[:], b1_d.ap()[:])
            nc.sync.dma_start(b2_sb[:], b2_d.ap()[:])
            wout_r = woutT_d.ap().rearrange("(k p) o -> p k o", p=128)
            nc.sync.dma_start(
                woutm_sb[:].rearrange("p (k o) -> p k o", k=KT2), wout_r)

            # --- DRAM bounce buffers: single AllReduce of the per-core
            #     w2tn partials (each core PE-matvecs M2T @ its local t1) ---
            wb = dram.tile([H], f32)
            wr = dram.tile([H], f32)

            x2 = []
            with tc.tile_pool(name="x0", bufs=1) as x0_pool, \
                 tc.tile_pool(name="w1", bufs=1) as w1_pool, \
                 tc.tile_pool(name="h1c", bufs=6 if FP8 else 3) as h1_pool, \
                 tc.tile_pool(name="nz1", bufs=3 if FP8 else 2) as nz1_pool, \
                 tc.tile_pool(name="habs", bufs=2) as habs_pool, \
                 tc.tile_pool(name="m2tp", bufs=1) as m2t_pool, \
                 tc.tile_pool(name="cm1", bufs=3 if FP8 else 2) as cm1_pool:

                # ---- interleaved x0/w1 loads so the first psum chain can
                #      start after ~1MB of DMA ----
                x0, w1 = [], []
                for kt in range(KP1):
                    x0t = x0_pool.tile([128, KX], MMDT, name=f"x0_{kt}")
                    nc.sync.dma_start(x0t[:], x0T_d.ap()[bass.ts(kt, 128), :])
                    x0.append(x0t)
                    w1t = w1_pool.tile([128, KW], MMDT, name=f"w1_{kt}")
                    nc.sync.dma_start(w1t[:], w1T_d.ap()[bass.ts(kt, 128), :])
                    w1.append(w1t)

                # --- W2 + M2 slices emitted AFTER x0/w1 so the startup
                #     loads win the DMA-queue race ---
                w2 = []
                for kt in range(KP2):
                    w2t = w2h_pool.tile([128, KW], MMDT, name=f"w2_{kt}")
                    wdma = (nc.scalar.dma_start if WQ_SCALAR
                            else nc.sync.dma_start)
                    wdma(w2t[:], w2T_d.ap()[bass.ts(kt, 128), :])
                    w2.append(w2t)
                m2t = []
                for kt in range(MT):
                    m2tt = m2t_pool.tile([128, H], fp8, name=f"m2t_{kt}")
                    wdma = (nc.scalar.dma_start if WQ_SCALAR
                            else nc.sync.dma_start)
                    wdma(m2tt[:], m2T_d.ap()[bass.ts(kt, 128), :])
                    m2t.append(m2tt)

                # ---- L1: matmuls, tanh+bias, +noise (Pool), abs-reduce
                #      (DVE), x2 = (1+cm1)*h1 fused on DVE ----
                for mt in range(MT):
                    h1t = h1_pool.tile([128, BC], bf16, name="h1c")
                    pss = [psum_mm.tile([128, 512], f32, name="psmm")
                           for _ in range(NCH)]
                    for kt in range(KP1):
                        for nch in range(NCH):
                            nc.tensor.matmul(
                                pss[nch][:], mm_ap(w1[kt], bass.ts(mt, 128)),
                                mm_ap(x0[kt], bass.ts(nch, 512)),
                                start=(kt == 0), stop=(kt == KP1 - 1), **MMKW)
                    for nch in range(NCH):
                        nc.scalar.activation(h1t[:, bass.ts(nch, 512)],
                                             pss[nch][:], ACT.Tanh,
                                             bias=b1_sb[:, mt:mt + 1],
                                             scale=WSC)
                    nzt = nz1_pool.tile([128, BC], NDT, name="nz1")
                    nc.sync.dma_start(nzt[:], n1T_d.ap()[bass.ts(mt, 128), :])
                    eng_add = nc.gpsimd if POOL_ADD else nc.vector
                    if FP8:
                        eng_add.scalar_tensor_tensor(
                            out=h1t[:], in0=nzt[:], scalar=1.0 / NSC,
                            in1=h1t[:], op0=ALU.mult, op1=ALU.add)
                    else:
                        eng_add.tensor_tensor(out=h1t[:], in0=h1t[:],
                                              in1=nzt[:], op=ALU.add)
                    habs = habs_pool.tile([128, BC], bf16, name="habs")
                    nc.scalar.activation(habs[:], h1t[:], ACT.Abs,
                                         accum_out=t1_sb[:, mt:mt + 1])
                    cmt = cm1_pool.tile([128, BC], NDT, name="cm1")
                    nc.sync.dma_start(cmt[:], cm1T_d.ap()[bass.ts(mt, 128), :])
                    if FP8:
                        if mt % 2 == 0:
                            x2.append(xx_pool.tile([128, KX], fp8,
                                                   name=f"x2_{mt // 2}"))
                        x2dst = x2[mt // 2][:, bass.ts(mt % 2, BC)]
                    else:
                        x2.append(xx_pool.tile([128, BC], bf16,
                                               name=f"x2_{mt}"))
                        x2dst = x2[mt][:]
                    nc.vector.scalar_tensor_tensor(
                        out=x2dst, in0=cmt[:], scalar=NSC, in1=h1t[:],
                        op0=ALU.add, op1=ALU.mult)

                # ---- thermal round: export t1 partial, AllReduce t1,
                #      fused DVE matvec slice of M2@t1, AllGather ----
                t1p_r = t1p_d.ap().rearrange("(m p) -> p m", p=128)
                nc.sync.dma_start(t1p_r, t1_sb[:])
                # local w2tn partial = M2T.T @ t1_local on the PE, then ONE
                # 8KB AllReduce sums the partials across cores
                nc.vector.tensor_scalar_mul(t1b_sb[:], t1_sb[:], 0.5)
                for ch in range(4):
                    psmv = psum_mm.tile([1, 512], f32, name="psmm")
                    for kt in range(MT):
                        nc.tensor.matmul(
                            psmv[:], t1b_sb[:, kt:kt + 1],
                            m2t[kt][:, bass.ts(ch, 512)],
                            start=(kt == 0), stop=(kt == MT - 1))
                    nc.scalar.mul(w2tnp_sb[:, bass.ts(ch, 512)], psmv[:],
                                  1.0 / 16384.0)
                wb_r = wb.rearrange("(a m) -> a m", a=1)
                nc.sync.dma_start(wb_r, w2tnp_sb[:])
                nc.gpsimd.collective_compute(
                    "AllReduce", ALU.add, replica_groups=RG,
                    ins=[wb.opt()], outs=[wr.opt()])
                wr_r = wr.rearrange("(m p) -> p m", p=128)
                nc.sync.dma_start(w2tn_sb[:], wr_r)
                nc.vector.tensor_tensor(out=bias2_sb[:], in0=b2_sb[:],
                                        in1=w2tn_sb[:], op=ALU.add)

            # ---- L2: full-K psum accumulation, Act-copy eviction to bf16
            #      z2; tail (tanh+bias2, +noise, reduce, x3, y-matmul)
            #      emitted per m-half so it pipelines under the matmuls ----
            with tc.tile_pool(name="z2p", bufs=1) as z2_pool, \
                 tc.tile_pool(name="x3c", bufs=1) as x3_pool, \
                 tc.tile_pool(name="nz2", bufs=3) as nz2_pool, \
                 tc.tile_pool(name="cm2", bufs=3) as cm2_pool, \
                 tc.tile_pool(name="ysb", bufs=1) as y_pool:

                z2 = [None] * MT
                x3 = [None] * MT
                pso = [psum_out.tile([DOUT, 512], f32, name="pso")
                       for _ in range(NCH)]

                def emit_mains(mts):
                    for mt in mts:
                        z2t = z2_pool.tile([128, BC], bf16, name=f"z2_{mt}")
                        z2[mt] = z2t
                        for nch in range(NCH):
                            ps = psum_mm.tile([128, 512], f32, name="psmm")
                            for kt in range(KP2):
                                nc.tensor.matmul(
                                    ps[:], mm_ap(w2[kt], bass.ts(mt, 128)),
                                    mm_ap(x2[kt], bass.ts(nch, 512)),
                                    start=(kt == 0), stop=(kt == KP2 - 1),
                                    **MMKW)
                            nc.scalar.mul(z2t[:, bass.ts(nch, 512)], ps[:],
                                          WSC2)

                def emit_tail(mts):
                    for mt in mts:
                        z2t = z2[mt]
                        for nch in range(NCH):
                            dst = z2t[:, bass.ts(nch, 512)]
                            nc.scalar.activation(dst, dst, ACT.Tanh,
                                                 bias=bias2_sb[:, mt:mt + 1])
                        nzt = nz2_pool.tile([128, BC], NDT, name="nz2")
                        nc.sync.dma_start(nzt[:],
                                          n2T_d.ap()[bass.ts(mt, 128), :])
                        eng_add = nc.gpsimd if POOL_ADD else nc.vector
                        if FP8:
                            eng_add.scalar_tensor_tensor(
                                out=z2t[:], in0=nzt[:], scalar=1.0 / NSC,
                                in1=z2t[:], op0=ALU.mult, op1=ALU.add)
                        else:
                            eng_add.tensor_tensor(out=z2t[:], in0=z2t[:],
                                                  in1=nzt[:], op=ALU.add)
                        nc.vector.tensor_reduce(
                            out=t2_sb[:, mt:mt + 1], in_=z2t[:],
                            axis=mybir.AxisListType.X, op=ALU.add,
                            apply_absolute_value=True)
                        cmt = cm2_pool.tile([128, BC], NDT, name="cm2")
                        nc.sync.dma_start(cmt[:],
                                          cm2T_d.ap()[bass.ts(mt, 128), :])
                        x3t = x3_pool.tile([128, BC], bf16, name=f"x3_{mt}")
                        nc.vector.scalar_tensor_tensor(
                            out=x3t[:], in0=cmt[:], scalar=NSC, in1=z2t[:],
                            op0=ALU.add, op1=ALU.mult)
                        x3[mt] = x3t

                def emit_ymm(kts):
                    for kt in kts:
                        for nch in range(NCH):
                            nc.tensor.matmul(
                                pso[nch][:], woutm_sb[:, bass.ts(kt, DOUT)],
                                x3[kt][:, bass.ts(nch, 512)],
                                start=(kt == 0), stop=(kt == KT2 - 1))

                HALF = MT // 2
                emit_mains(range(0, HALF))
                emit_tail(range(0, HALF))          # gated on bias2; Act does
                emit_mains(range(HALF, MT))        # these tanhs between the
                emit_ymm(range(0, HALF))           # second-half evictions
                emit_tail(range(HALF, MT))
                emit_ymm(range(HALF, MT))

                # ---- t2 partial export + output eviction ----
                t2p_r = t2p_d.ap().rearrange("(m p) -> p m", p=128)
                nc.sync.dma_start(t2p_r, t2_sb[:])
                y_sb = y_pool.tile([DOUT, BC], f32, name="ysb")
                for nch in range(NCH):
                    nc.scalar.copy(y_sb[:, bass.ts(nch, 512)], pso[nch][:])
                nc.sync.dma_start(y_d.ap()[:], y_sb[:])
                if DEBUG:
                    for nm, t in [("d_t1", t1_sb), ("d_w2tn", w2tn_sb),
                                  ("d_bias2", bias2_sb)]:
                        nc.sync.dma_start(dbg_d[nm].ap()[:], t[:])

    nc.finalize()
    _NC_CACHE[(scheme, POOL_ADD, USE_TTR, WQ_SCALAR)] = nc
    return nc


def _prep_inputs(x, W1, b1, W2, b2, Wout, bout, scheme):
    consts = _gen_constants()
    import ml_dtypes
    f32 = np.float32
    bf = np.dtype(ml_dtypes.bfloat16)

    FP8 = scheme == "fp8"
    f8 = np.dtype(ml_dtypes.float8_e4m3)

    def pair_rows(A):
        """[R, C] -> [R//2, 2C]: row (ktp*128+p) = [A[ktp*256+p], A[ktp*256+128+p]]
        — the DoubleRow paired-k layout."""
        R, C = A.shape
        return np.ascontiguousarray(
            A.reshape(R // 256, 2, 128, C).transpose(0, 2, 1, 3)
            .reshape(R // 2, 2 * C))

    x0Tf = np.tanh(np.asarray(x, f32)).T                       # [D, B] f32
    if FP8:
        W1T = pair_rows(np.asarray(W1, f32).T * 8).astype(f8)  # [D/2, 2H]
        W2T = pair_rows(np.asarray(W2, f32).T * 8).astype(f8)  # [H/2, 2H]
    else:
        W1T = np.asarray(W1, f32).T.astype(bf)                 # [D, H]
        W2T = np.asarray(W2, f32).T.astype(bf)                 # [H, H]
    WoutT = (np.asarray(Wout, f32).T
             * (1.0 / 64.0 if FP8 else 1.0)).astype(bf)        # [H, 2]
    Ks64 = consts["Ks"].astype(np.float64)
    M2T = np.ascontiguousarray(
        (np.asarray(W2, np.float64) @ Ks64).T * 32768.0).astype(f8)  # [H, H]
    b1s = np.ascontiguousarray(np.asarray(b1, f32).reshape(MT, 128).T)
    b2s = np.ascontiguousarray(np.asarray(b2, f32).reshape(MT, 128).T)
    ndt = f8 if FP8 else bf
    nsc = np.float32(64.0 if FP8 else 1.0)
    n1T = (consts["noise0T"] * nsc).astype(ndt)
    cm1T = (consts["cmm0T"] * nsc).astype(ndt)
    n2T = (consts["noise1T"] * nsc).astype(ndt)
    cm2T = (consts["cmm1T"] * nsc).astype(ndt)

    in_maps = []
    for c in range(N_CORES):
        bs = slice(c * BC, (c + 1) * BC)
        fs = slice(c * SL, (c + 1) * SL)
        if FP8:
            x0c = pair_rows(np.ascontiguousarray(x0Tf[:, bs])).astype(f8)
        else:
            x0c = np.ascontiguousarray(x0Tf[:, bs]).astype(bf)
        in_maps.append({
            "x0T": x0c,
            "w1T": W1T,
            "w2T": W2T,
            "woutT": WoutT,
            "m2T": M2T,
            "n1T": np.ascontiguousarray(n1T[:, bs]),
            "cm1T": np.ascontiguousarray(cm1T[:, bs]),
            "n2T": np.ascontiguousarray(n2T[:, bs]),
            "cm2T": np.ascontiguousarray(cm2T[:, bs]),
            "b1s": b1s,
            "b2s": b2s,
        })
    return in_maps


def kernel(x, W1, b1, W2, b2, Wout, bout, **kw):
    scheme = SCHEME
    nc = _build_nc(scheme)
    in_maps = _prep_inputs(x, W1, b1, W2, b2, Wout, bout, scheme)
    res = bass_utils.run_bass_kernel_spmd(nc, in_maps,
                                          core_ids=list(range(N_CORES)))
    # host adds the batch-constant output offset: Wout@tn2 + bout, with
    # tn2 = 0.7*Ks@t1 + Ks@t2 and t1/t2 summed from per-core partials
    consts = _gen_constants()
    MoutR = (np.asarray(Wout, np.float64)
             @ consts["Ks"].astype(np.float64))
    t1f = np.zeros(H, np.float64)
    t2f = np.zeros(H, np.float64)
    for c in range(N_CORES):
        t1f += res.results[c]["t1p"].astype(np.float64)
        t2f += res.results[c]["t2p"].astype(np.float64)
    obias = (0.7 * (MoutR @ t1f) + MoutR @ t2f
             + np.asarray(bout, np.float64)).astype(np.float32)
    out = np.empty((B, DOUT), np.float32)
    for c in range(N_CORES):
        out[c * BC:(c + 1) * BC, :] = res.results[c]["y"].T + obias[None, :]
    return out


# revision 19
# speedup vs baseline: 1.3036x; 1.1125x over previous
"""PhotonicNeuralNetwork TRN2 kernel — 8-core data-parallel over batch, v2.

Architecture (all feature-major / transposed space, no device transposes):
  h.T = W @ x.T per layer; batch sharded 1024 rows/core across 8 cores.
  All matmul operands are bf16, host-precast (weights, tanh(x), noise, cm)
  — halves DMA vs fp32 and removes all on-device weight casts.

  Thermal path needs only ONE collective round: the layer-2 bias
  w2tn = W2 @ tn1 = (W2 @ Ks) @ t1 = M2 @ t1 must be applied inside
  tanh(z2 + b2 + w2tn) on device.  Each core: AllReduce t1 (8KB) ->
  computes a 256-row slice of M2 @ t1 with a fused DVE multiply-reduce ->
  AllGather slices (1KB/core).  Everything else thermal-related is
  batch-constant and applied on HOST: obias = Wout @ tn2 + bout with
  tn2 = 0.7*Ks@t1 + Ks@t2, where t1/t2 are host-summed per-core partial
  abs-sums (plain DMA exports, no second collective round).
  Dropped terms (validated 8.9e-5 rel in f64 sim): W2@(tn1*cm1),
  Wout@(tn2*cm2).

  Engine split: PE matmuls; Act tanh + PSUM evictions; Pool noise adds;
  DVE abs-reduces + fused (1+cm)*h ops + the thermal matvec.  L2 tail is
  emitted in m-halves interleaved with the L2 matmul stream so the
  tanh/noise/reduce/x3/y-matmul pipeline hides under PE compute.
"""
import os
import sys
import subprocess
import tempfile

import numpy as np

for _p in ("/opt/trn_rl_repo", "/root/.axon_site/_ro/trn_rl_repo"):
    if _p not in sys.path and os.path.isdir(_p):
        sys.path.append(_p)

import concourse.bass as bass  # noqa: E402
import concourse.mybir as mybir  # noqa: E402
import concourse.tile as tile  # noqa: E402
from concourse import bass_utils, bacc  # noqa: E402

# Problem shapes (hardcoded per contract)
B, D, H, DOUT = 8192, 1024, 2048, 2
N_CORES = 8
BC = B // N_CORES          # 1024 batch columns per core
SL = H // N_CORES          # 256 features per core for the matvec slice
KT1 = D // 128             # 8
MT = H // 128              # 16
KT2 = H // 128             # 16
NCH = BC // 512            # 2 psum chunks of 512
TN_SCALE = 0.05 * 0.3 * 0.05   # 7.5e-4, folded into Ks

SCHEME = os.environ.get("PNN_SCHEME", "fp8")
# bisection toggles (default = full-featured)
POOL_ADD = os.environ.get("PNN_POOL_ADD", "0") == "1"   # noise adds on Pool
USE_TTR = os.environ.get("PNN_TTR", "0") == "1"         # fused mult+reduce
WQ_SCALAR = os.environ.get("PNN_WQ", "scalar") == "scalar"  # w2 DMA queue

_CONSTS = {}
_NC_CACHE = {}


def _gen_constants():
    """Noise/coherence constants + crosstalk kernel, bit-exact with the
    reference's jax-on-CPU PRNG. Runs in a subprocess pinned to the CPU
    backend so the parent's jax platform config doesn't matter."""
    if _CONSTS:
        return _CONSTS
    script = r"""
import sys
import jax
jax.config.update("jax_platforms", "cpu")
import numpy as np
import jax.numpy as jnp
outdir = sys.argv[1]
B, H = 8192, 2048
nkey = jax.random.key(42)
for li in range(2):
    k_noise = jax.random.fold_in(nkey, 2 * li)
    k_phase = jax.random.fold_in(nkey, 2 * li + 1)
    n = jax.random.normal(k_noise, (B, H), jnp.float32) * np.float32(0.02)
    ph = jax.random.normal(k_phase, (B, H), jnp.float32) * np.float32(0.03)
    cm = (jnp.cos(ph) - np.float32(1.0)) * np.float32(0.03)
    np.save(f"{outdir}/n{li}.npy", np.asarray(n).T.copy())
    np.save(f"{outdir}/cm{li}.npy", np.asarray(cm).T.copy())
idx = jnp.arange(H, dtype=jnp.float32)
dist = jnp.abs(idx[:, None] - idx[None, :])
K = jnp.where(dist > 0, 1.0 / (dist * dist), 0.0)
np.save(f"{outdir}/K.npy", np.asarray(K))
"""
    with tempfile.TemporaryDirectory() as td:
        env = dict(os.environ)
        env["JAX_PLATFORMS"] = "cpu"
        env.pop("JAX_PLATFORM_NAME", None)
        subprocess.run([sys.executable, "-c", script, td], env=env, check=True,
                       capture_output=True)
        for li in range(2):
            _CONSTS[f"noise{li}T"] = np.load(f"{td}/n{li}.npy")   # [H, B] fp32
            _CONSTS[f"cmm{li}T"] = np.load(f"{td}/cm{li}.npy")    # [H, B] fp32
        K = np.load(f"{td}/K.npy")                                # [H, H] fp32
    _CONSTS["Ks"] = (K.astype(np.float64) * TN_SCALE).astype(np.float32)
    return _CONSTS


def _build_nc(scheme):
    key = (scheme, POOL_ADD, USE_TTR, WQ_SCALAR)
    if key in _NC_CACHE:
        return _NC_CACHE[key]
    bf16 = mybir.dt.bfloat16
    fp8 = mybir.dt.float8e4
    f32 = mybir.dt.float32
    ACT = mybir.ActivationFunctionType
    ALU = mybir.AluOpType
    FP8 = scheme == "fp8"
    MMDT = fp8 if FP8 else bf16         # matmul operand dtype (hidden layers)
    DR = mybir.MatmulPerfMode.DoubleRow if FP8 else None
    KP1 = KT1 // 2 if FP8 else KT1      # lhs k-tile count layer 1
    KP2 = KT2 // 2 if FP8 else KT2      # lhs k-tile count layer 2
    WSC = 0.125 if FP8 else 1.0         # un-scale for the x8 fp8 weights
    WSC2 = 0.125 / 64.0 if FP8 else 1.0  # also undo the x64 cm/x2 scale
    KW = 2 * H if FP8 else H            # weight tile free size
    KX = 2 * BC if FP8 else BC          # x tile free size

    nc = bacc.Bacc(trn_type="TRN2", target_bir_lowering=False, debug=False,
                   num_devices=N_CORES)

    x0T_d = nc.dram_tensor("x0T", [KP1 * 128, KX], MMDT, kind="ExternalInput")
    w1T_d = nc.dram_tensor("w1T", [KP1 * 128, KW], MMDT, kind="ExternalInput")
    w2T_d = nc.dram_tensor("w2T", [KP2 * 128, KW], MMDT, kind="ExternalInput")
    woutT_d = nc.dram_tensor("woutT", [H, DOUT], bf16, kind="ExternalInput")
    m2T_d = nc.dram_tensor("m2T", [H, H], fp8, kind="ExternalInput")
    NDT = fp8 if FP8 else bf16          # noise/cm storage dtype
    NSC = 64.0 if FP8 else 1.0          # noise/cm host-side scale
    n1T_d = nc.dram_tensor("n1T", [H, BC], NDT, kind="ExternalInput")
    cm1T_d = nc.dram_tensor("cm1T", [H, BC], NDT, kind="ExternalInput")
    n2T_d = nc.dram_tensor("n2T", [H, BC], NDT, kind="ExternalInput")
    cm2T_d = nc.dram_tensor("cm2T", [H, BC], NDT, kind="ExternalInput")
    b1_d = nc.dram_tensor("b1s", [128, MT], f32, kind="ExternalInput")
    b2_d = nc.dram_tensor("b2s", [128, MT], f32, kind="ExternalInput")
    y_d = nc.dram_tensor("y", [DOUT, BC], f32, kind="ExternalOutput")
    t1p_d = nc.dram_tensor("t1p", [H], f32, kind="ExternalOutput")
    h2d_d = nc.dram_tensor("h2d", [H, BC], bf16, kind="ExternalOutput")
    DEBUG = os.environ.get("PNN_DEBUG", "0") == "1"
    if DEBUG:
        dbg_d = {n: nc.dram_tensor(n, [128, MT], f32, kind="ExternalOutput")
                 for n in ["d_t1", "d_w2tn", "d_bias2"]}

    def mm_ap(t, sl):
        if FP8:
            return t[:].rearrange("p (i f) -> p i f", i=2)[:, :, sl]
        return t[:, sl]

    MMKW = {"perf_mode": DR} if FP8 else {}

    RG = [list(range(N_CORES))]
    with tile.TileContext(nc) as tc:
        with tc.tile_pool(name="dram", bufs=1, space="DRAM") as dram, \
             tc.tile_pool(name="smalls", bufs=1) as smalls, \
             tc.tile_pool(name="psum_mm", bufs=6, space="PSUM") as psum_mm, \
             tc.tile_pool(name="psum_out", bufs=2, space="PSUM") as psum_out, \
             tc.tile_pool(name="xx", bufs=1) as xx_pool, \
             tc.tile_pool(name="w2h", bufs=1) as w2h_pool, \
             tc.tile_pool(name="w2h2", bufs=1) as _unused_pool:

            # --- small persistent tiles ---
            b1_sb = smalls.tile([128, MT], f32)
            b2_sb = smalls.tile([128, MT], f32)
            t1_sb = smalls.tile([128, MT], f32)
            t2_sb = smalls.tile([128, MT], f32)
            w2tn_sb = smalls.tile([128, MT], f32)
            bias2_sb = smalls.tile([128, MT], f32)
            t1b_sb = smalls.tile([128, MT], fp8)
            w2tnp_sb = smalls.tile([1, H], f32)
            woutm_sb = smalls.tile([128, KT2 * DOUT], bf16)

            nc.sync.dma_start(b1_sb[:], b1_d.ap()[:])
            nc.sync.dma_start(b2_sb[:], b2_d.ap()[:])
            wout_r = woutT_d.ap().rearrange("(k p) o -> p k o", p=128)
            nc.sync.dma_start(
                woutm_sb[:].rearrange("p (k o) -> p k o", k=KT2), wout_r)

            # --- DRAM bounce buffers: single AllReduce of the per-core
            #     w2tn partials (each core PE-matvecs M2T @ its local t1) ---
            wb = dram.tile([H], f32)
            wr = dram.tile([H], f32)

            x2 = []
            with tc.tile_pool(name="x0", bufs=1) as x0_pool, \
                 tc.tile_pool(name="w1", bufs=1) as w1_pool, \
                 tc.tile_pool(name="h1c", bufs=6 if FP8 else 3) as h1_pool, \
                 tc.tile_pool(name="nz1", bufs=3 if FP8 else 2) as nz1_pool, \
                 tc.tile_pool(name="habs", bufs=2) as habs_pool, \
                 tc.tile_pool(name="m2tp", bufs=1) as m2t_pool, \
                 tc.tile_pool(name="cm1", bufs=3 if FP8 else 2) as cm1_pool:

                # ---- interleaved x0/w1 loads so the first psum chain can
                #      start after ~1MB of DMA ----
                x0, w1 = [], []
                for kt in range(KP1):
                    x0t = x0_pool.tile([128, KX], MMDT, name=f"x0_{kt}")
                    nc.sync.dma_start(x0t[:], x0T_d.ap()[bass.ts(kt, 128), :])
                    x0.append(x0t)
                    w1t = w1_pool.tile([128, KW], MMDT, name=f"w1_{kt}")
                    nc.sync.dma_start(w1t[:], w1T_d.ap()[bass.ts(kt, 128), :])
                    w1.append(w1t)

                # --- W2 + M2 slices emitted AFTER x0/w1 so the startup
                #     loads win the DMA-queue race ---
                w2 = []
                for kt in range(KP2):
                    w2t = w2h_pool.tile([128, KW], MMDT, name=f"w2_{kt}")
                    wdma = (nc.scalar.dma_start if WQ_SCALAR
                            else nc.sync.dma_start)
                    wdma(w2t[:], w2T_d.ap()[bass.ts(kt, 128), :])
                    w2.append(w2t)
                m2t = []
                for kt in range(MT):
                    m2tt = m2t_pool.tile([128, H], fp8, name=f"m2t_{kt}")
                    wdma = (nc.scalar.dma_start if WQ_SCALAR
                            else nc.sync.dma_start)
                    wdma(m2tt[:], m2T_d.ap()[bass.ts(kt, 128), :])
                    m2t.append(m2tt)

                # ---- L1: matmuls, tanh+bias, +noise (Pool), abs-reduce
                #      (DVE), x2 = (1+cm1)*h1 fused on DVE ----
                for mt in range(MT):
                    h1t = h1_pool.tile([128, BC], bf16, name="h1c")
                    pss = [psum_mm.tile([128, 512], f32, name="psmm")
                           for _ in range(NCH)]
                    for kt in range(KP1):
                        for nch in range(NCH):
                            nc.tensor.matmul(
                                pss[nch][:], mm_ap(w1[kt], bass.ts(mt, 128)),
                                mm_ap(x0[kt], bass.ts(nch, 512)),
                                start=(kt == 0), stop=(kt == KP1 - 1), **MMKW)
                    for nch in range(NCH):
                        nc.scalar.activation(h1t[:, bass.ts(nch, 512)],
                                             pss[nch][:], ACT.Tanh,
                                             bias=b1_sb[:, mt:mt + 1],
                                             scale=WSC)
                    nzt = nz1_pool.tile([128, BC], NDT, name="nz1")
                    nc.sync.dma_start(nzt[:], n1T_d.ap()[bass.ts(mt, 128), :])
                    eng_add = nc.gpsimd if POOL_ADD else nc.vector
                    if FP8:
                        eng_add.scalar_tensor_tensor(
                            out=h1t[:], in0=nzt[:], scalar=1.0 / NSC,
                            in1=h1t[:], op0=ALU.mult, op1=ALU.add)
                    else:
                        eng_add.tensor_tensor(out=h1t[:], in0=h1t[:],
                                              in1=nzt[:], op=ALU.add)
                    habs = habs_pool.tile([128, BC], bf16, name="habs")
                    nc.scalar.activation(habs[:], h1t[:], ACT.Abs,
                                         accum_out=t1_sb[:, mt:mt + 1])
                    cmt = cm1_pool.tile([128, BC], NDT, name="cm1")
                    nc.sync.dma_start(cmt[:], cm1T_d.ap()[bass.ts(mt, 128), :])
                    if FP8:
                        if mt % 2 == 0:
                            x2.append(xx_pool.tile([128, KX], fp8,
                                                   name=f"x2_{mt // 2}"))
                        x2dst = x2[mt // 2][:, bass.ts(mt % 2, BC)]
                    else:
                        x2.append(xx_pool.tile([128, BC], bf16,
                                               name=f"x2_{mt}"))
                        x2dst = x2[mt][:]
                    nc.vector.scalar_tensor_tensor(
                        out=x2dst, in0=cmt[:], scalar=NSC, in1=h1t[:],
                        op0=ALU.add, op1=ALU.mult)

                # ---- thermal round: export t1 partial, AllReduce t1,
                #      fused DVE matvec slice of M2@t1, AllGather ----
                t1p_r = t1p_d.ap().rearrange("(m p) -> p m", p=128)
                nc.sync.dma_start(t1p_r, t1_sb[:])
                # local w2tn partial = M2T.T @ t1_local on the PE, then ONE
                # 8KB AllReduce sums the partials across cores
                nc.vector.tensor_scalar_mul(t1b_sb[:], t1_sb[:], 0.5)
                for ch in range(4):
                    psmv = psum_mm.tile([1, 512], f32, name="psmm")
                    for kt in range(MT):
                        nc.tensor.matmul(
                            psmv[:], t1b_sb[:, kt:kt + 1],
                            m2t[kt][:, bass.ts(ch, 512)],
                            start=(kt == 0), stop=(kt == MT - 1))
                    nc.scalar.mul(w2tnp_sb[:, bass.ts(ch, 512)], psmv[:],
                                  1.0 / 16384.0)
                wb_r = wb.rearrange("(a m) -> a m", a=1)
                nc.sync.dma_start(wb_r, w2tnp_sb[:])
                nc.gpsimd.collective_compute(
                    "AllReduce", ALU.add, replica_groups=RG,
                    ins=[wb.opt()], outs=[wr.opt()])
                wr_r = wr.rearrange("(m p) -> p m", p=128)
                nc.sync.dma_start(w2tn_sb[:], wr_r)
                nc.vector.tensor_tensor(out=bias2_sb[:], in0=b2_sb[:],
                                        in1=w2tn_sb[:], op=ALU.add)

            # ---- L2: full-K psum accumulation, Act-copy eviction to bf16
            #      z2; tail (tanh+bias2, +noise, reduce, x3, y-matmul)
            #      emitted per m-half so it pipelines under the matmuls ----
            with tc.tile_pool(name="z2p", bufs=1) as z2_pool, \
                 tc.tile_pool(name="x3c", bufs=1) as x3_pool, \
                 tc.tile_pool(name="nz2", bufs=3) as nz2_pool, \
                 tc.tile_pool(name="cm2", bufs=3) as cm2_pool, \
                 tc.tile_pool(name="ysb", bufs=1) as y_pool:

                z2 = [None] * MT
                x3 = [None] * MT
                pso = [psum_out.tile([DOUT, 512], f32, name="pso")
                       for _ in range(NCH)]

                def emit_mains(mts):
                    for mt in mts:
                        z2t = z2_pool.tile([128, BC], bf16, name=f"z2_{mt}")
                        z2[mt] = z2t
                        for nch in range(NCH):
                            ps = psum_mm.tile([128, 512], f32, name="psmm")
                            for kt in range(KP2):
                                nc.tensor.matmul(
                                    ps[:], mm_ap(w2[kt], bass.ts(mt, 128)),
                                    mm_ap(x2[kt], bass.ts(nch, 512)),
                                    start=(kt == 0), stop=(kt == KP2 - 1),
                                    **MMKW)
                            nc.scalar.mul(z2t[:, bass.ts(nch, 512)], ps[:],
                                          WSC2)

                def emit_tail(mts):
                    for mt in mts:
                        z2t = z2[mt]
                        for nch in range(NCH):
                            dst = z2t[:, bass.ts(nch, 512)]
                            nc.scalar.activation(dst, dst, ACT.Tanh,
                                                 bias=bias2_sb[:, mt:mt + 1])
                        nzt = nz2_pool.tile([128, BC], NDT, name="nz2")
                        nc.sync.dma_start(nzt[:],
                                          n2T_d.ap()[bass.ts(mt, 128), :])
                        eng_add = nc.gpsimd if POOL_ADD else nc.vector
                        if FP8:
                            eng_add.scalar_tensor_tensor(
                                out=z2t[:], in0=nzt[:], scalar=1.0 / NSC,
                                in1=z2t[:], op0=ALU.mult, op1=ALU.add)
                        else:
                            eng_add.tensor_tensor(out=z2t[:], in0=z2t[:],
                                                  in1=nzt[:], op=ALU.add)
                        nc.sync.dma_start(h2d_d.ap()[bass.ts(mt, 128), :],
                                          z2t[:])
                        cmt = cm2_pool.tile([128, BC], NDT, name="cm2")
                        nc.sync.dma_start(cmt[:],
                                          cm2T_d.ap()[bass.ts(mt, 128), :])
                        x3t = x3_pool.tile([128, BC], bf16, name=f"x3_{mt}")
                        nc.vector.scalar_tensor_tensor(
                            out=x3t[:], in0=cmt[:], scalar=NSC, in1=z2t[:],
                            op0=ALU.add, op1=ALU.mult)
                        x3[mt] = x3t

                def emit_ymm(kts):
                    for kt in kts:
                        for nch in range(NCH):
                            nc.tensor.matmul(
                                pso[nch][:], woutm_sb[:, bass.ts(kt, DOUT)],
                                x3[kt][:, bass.ts(nch, 512)],
                                start=(kt == 0), stop=(kt == KT2 - 1))

                HALF = MT // 2
                emit_mains(range(0, HALF))
                emit_tail(range(0, HALF))          # gated on bias2; Act does
                emit_mains(range(HALF, MT))        # these tanhs between the
                emit_ymm(range(0, HALF))           # second-half evictions
                emit_tail(range(HALF, MT))
                emit_ymm(range(HALF, MT))

                # ---- output eviction (t2 is host-summed from h2d) ----
                y_sb = y_pool.tile([DOUT, BC], f32, name="ysb")
                for nch in range(NCH):
                    nc.scalar.copy(y_sb[:, bass.ts(nch, 512)], pso[nch][:])
                nc.sync.dma_start(y_d.ap()[:], y_sb[:])
                if DEBUG:
                    for nm, t in [("d_t1", t1_sb), ("d_w2tn", w2tn_sb),
                                  ("d_bias2", bias2_sb)]:
                        nc.sync.dma_start(dbg_d[nm].ap()[:], t[:])

    nc.finalize()
    _NC_CACHE[(scheme, POOL_ADD, USE_TTR, WQ_SCALAR)] = nc
    return nc


def _prep_inputs(x, W1, b1, W2, b2, Wout, bout, scheme):
    consts = _gen_constants()
    import ml_dtypes
    f32 = np.float32
    bf = np.dtype(ml_dtypes.bfloat16)

    FP8 = scheme == "fp8"
    f8 = np.dtype(ml_dtypes.float8_e4m3)

    def pair_rows(A):
        """[R, C] -> [R//2, 2C]: row (ktp*128+p) = [A[ktp*256+p], A[ktp*256+128+p]]
        — the DoubleRow paired-k layout."""
        R, C = A.shape
        return np.ascontiguousarray(
            A.reshape(R // 256, 2, 128, C).transpose(0, 2, 1, 3)
            .reshape(R // 2, 2 * C))

    x0Tf = np.tanh(np.asarray(x, f32)).T                       # [D, B] f32
    if FP8:
        W1T = pair_rows(np.asarray(W1, f32).T * 8).astype(f8)  # [D/2, 2H]
        W2T = pair_rows(np.asarray(W2, f32).T * 8).astype(f8)  # [H/2, 2H]
    else:
        W1T = np.asarray(W1, f32).T.astype(bf)                 # [D, H]
        W2T = np.asarray(W2, f32).T.astype(bf)                 # [H, H]
    WoutT = (np.asarray(Wout, f32).T
             * (1.0 / 64.0 if FP8 else 1.0)).astype(bf)        # [H, 2]
    Ks64 = consts["Ks"].astype(np.float64)
    M2T = np.ascontiguousarray(
        (np.asarray(W2, np.float64) @ Ks64).T * 32768.0).astype(f8)  # [H, H]
    b1s = np.ascontiguousarray(np.asarray(b1, f32).reshape(MT, 128).T)
    b2s = np.ascontiguousarray(np.asarray(b2, f32).reshape(MT, 128).T)
    ndt = f8 if FP8 else bf
    nsc = np.float32(64.0 if FP8 else 1.0)
    n1T = (consts["noise0T"] * nsc).astype(ndt)
    cm1T = (consts["cmm0T"] * nsc).astype(ndt)
    n2T = (consts["noise1T"] * nsc).astype(ndt)
    cm2T = (consts["cmm1T"] * nsc).astype(ndt)

    in_maps = []
    for c in range(N_CORES):
        bs = slice(c * BC, (c + 1) * BC)
        fs = slice(c * SL, (c + 1) * SL)
        if FP8:
            x0c = pair_rows(np.ascontiguousarray(x0Tf[:, bs])).astype(f8)
        else:
            x0c = np.ascontiguousarray(x0Tf[:, bs]).astype(bf)
        in_maps.append({
            "x0T": x0c,
            "w1T": W1T,
            "w2T": W2T,
            "woutT": WoutT,
            "m2T": M2T,
            "n1T": np.ascontiguousarray(n1T[:, bs]),
            "cm1T": np.ascontiguousarray(cm1T[:, bs]),
            "n2T": np.ascontiguousarray(n2T[:, bs]),
            "cm2T": np.ascontiguousarray(cm2T[:, bs]),
            "b1s": b1s,
            "b2s": b2s,
        })
    return in_maps


def kernel(x, W1, b1, W2, b2, Wout, bout, **kw):
    scheme = SCHEME
    nc = _build_nc(scheme)
    in_maps = _prep_inputs(x, W1, b1, W2, b2, Wout, bout, scheme)
    res = bass_utils.run_bass_kernel_spmd(nc, in_maps,
                                          core_ids=list(range(N_CORES)))
    # host adds the batch-constant output offset: Wout@tn2 + bout, with
    # tn2 = 0.7*Ks@t1 + Ks@t2 and t1/t2 summed from per-core partials
    consts = _gen_constants()
    MoutR = (np.asarray(Wout, np.float64)
             @ consts["Ks"].astype(np.float64))
    t1f = np.zeros(H, np.float64)
    t2f = np.zeros(H, np.float64)
    for c in range(N_CORES):
        t1f += res.results[c]["t1p"].astype(np.float64)
        t2f += np.abs(res.results[c]["h2d"].astype(np.float64)).sum(axis=1)
    obias = (0.7 * (MoutR @ t1f) + MoutR @ t2f
             + np.asarray(bout, np.float64)).astype(np.float32)
    out = np.empty((B, DOUT), np.float32)
    for c in range(N_CORES):
        out[c * BC:(c + 1) * BC, :] = res.results[c]["y"].T + obias[None, :]
    return out
